# revision 25
# baseline (speedup 1.0000x reference)
"""Trainium2 Bass kernel for nn_Model_39676907883957 (dense_transformer).

Math (per batch element b, with S = D = N = 2048):
    q = Xq @ Wq^T + bq            # [S, D]
    kT = Wk @ Xk^T + bk[:, None]  # [D, S]  (k projected directly in transposed layout)
    v = Xv @ Wv^T + bv            # [S, D]
    scores[i, j] = sum_m q[m, i] * kT[m, j]          # q^T @ k^T
    attn = softmax_rows(scores)
    out[n, i] = sum_j v[j, n] * attn[i, j]           # == (attn @ v)^T

Sharding: data-parallel over batch, B=8 -> one batch element per NeuronCore.

Numerics: ALL matmuls single-pass fp16 with fp32 PSUM accumulation
(5 matmul units of 2048^3). With scale_factor=1.0 the logits have
std ~ sqrt(2048) ~ 45, so the softmax is near-argmax per row and the
output tolerates fp16-level logit error; simulated rel err ~3.7e-3
against the fp32 reference (gate 2e-2).

Dataflow per core (phases back-to-back on the tensor engine):
  - Projections run chunk-outer so each phase's first groups touch only
    chunk-0 of the moving (resident) operand, which is prefetched into a
    small persistent pool during the PREVIOUS phase; the remaining chunks
    stream in behind compute. This removes SBUF ring-aliasing stalls at
    phase boundaries.
  - kT is written directly into SBUF-resident tiles (no DRAM roundtrip);
    q and v stage via DRAM fp16.
  - attn row-blocks transpose into one resident attn^T tile via one
    batched SBUF->SBUF DMA xbar op per row-block (sync queue).
  - Queue split: sync = streamed stationary loads + transposes;
    gpsimd (SWDGE) = resident/moving loads; scalar = biases + all DRAM
    stores; so loads never queue behind stores.
"""

import numpy as np

import concourse.bass as bass
import concourse.bacc as bacc
import concourse.tile as tile
import concourse.mybir as mybir
from concourse.bass_utils import run_bass_kernel_spmd

B, S, D = 8, 2048, 2048
N = 2048                 # S == D
KT = N // 128            # 16 contraction tiles
NCHUNK = N // 512        # 4 free-dim chunks of 512
F16 = mybir.dt.float16
F32 = mybir.dt.float32
AX = mybir.AxisListType.X
EXP = mybir.ActivationFunctionType.Exp

_compiled = {}


def _build():
    nc = bacc.Bacc("TRN2", target_bir_lowering=False, debug=False)

    # ExternalInputs (per core). x* are host-transposed activations [d, s].
    xq = nc.dram_tensor("xq", [N, N], F16, kind="ExternalInput").ap()
    xk = nc.dram_tensor("xk", [N, N], F16, kind="ExternalInput").ap()
    xv = nc.dram_tensor("xv", [N, N], F16, kind="ExternalInput").ap()
    # host-transposed weights [d, e]
    wq = nc.dram_tensor("wq", [N, N], F16, kind="ExternalInput").ap()
    wk = nc.dram_tensor("wk", [N, N], F16, kind="ExternalInput").ap()
    wv = nc.dram_tensor("wv", [N, N], F16, kind="ExternalInput").ap()
    # biases: bqb/bvb broadcast across partitions [128, N]; bkp partition-major [128, 16]
    bqb = nc.dram_tensor("bqb", [128, N], F32, kind="ExternalInput").ap()
    bkp = nc.dram_tensor("bkp", [128, KT], F32, kind="ExternalInput").ap()
    bvb = nc.dram_tensor("bvb", [128, N], F32, kind="ExternalInput").ap()

    out = nc.dram_tensor("out", [N, N], F32, kind="ExternalOutput").ap()

    with tile.TileContext(nc, pool_alloc_mode="queue") as tc:
        with (
            tc.tile_pool(name="dram", bufs=1, space="DRAM") as dram,
            # persistent pools, allocated below everything else
            tc.tile_pool(name="stream", bufs=5) as stpool,   # stationary col-blocks
            tc.tile_pool(name="stout", bufs=2) as sopool,    # psum->SBUF staging
            tc.tile_pool(name="bias", bufs=1) as bpool,
            tc.tile_pool(name="pf", bufs=1) as pfpool,       # chunk-0 prefetch
            tc.tile_pool(name="ps", bufs=8, space="PSUM") as psum,
        ):
            q_s = dram.tile([N, N], F16, tag="q_s")
            v_s = dram.tile([N, N], F16, tag="v_s")
            pools = (stpool, sopool, bpool, pfpool, psum)

            with tc.tile_pool(name="kr", bufs=1) as krp:
                kr = [krp.tile([128, N], F16, tag=f"kr{k}", name=f"kr{k}")
                      for k in range(KT)]
                # startup: first stationary col-block alone on sync, then
                # chunk-0 weight prefetch spread over gpsimd+scalar
                ax0 = _load_kblock_b(nc, stpool, xq, 0, "st")
                pf_q = _prefetch_c0(nc, pfpool, wq, "pfq", spread=True)
                pf_k = _proj_rows(nc, tc, pools, xq, wq, bqb, q_s, pf_q,
                                  nxt=(xk, "pfk"), ax_pre=ax0)
                pf_v = _proj_cols_resident(nc, tc, pools, xk, wk, bkp, kr, pf_k,
                                           nxt=(wv, "pfv"))
                _proj_rows(nc, tc, pools, xv, wv, bvb, v_s, pf_v, nxt=None)

                with tc.tile_pool(name="att", bufs=1) as attp:
                    # attn^T resident: att[p, j_blk*N + i] = attn[i, j_blk*128+p]
                    att = attp.tile([128, KT * N], F16, tag="att", name="att")
                    _scores_softmax_t(nc, tc, pools, q_s, kr, att)
                    _attn_v(nc, tc, pools, v_s, att, out)

    nc.compile()
    return nc


def _prefetch_c0(nc, pfpool, src, tag, spread=False):
    """Load chunk 0 ([:, 0:512]) of all KT row-blocks of a [N, N] DRAM tensor
    into 16 [128, 512] tiles of the persistent prefetch pool."""
    engines = [nc.gpsimd, nc.scalar]
    ts = []
    for k in range(KT):
        t = pfpool.tile([128, 512], F16, tag=f"pf{k}", name=f"{tag}{k}")
        eng = engines[k % 2] if spread else nc.gpsimd
        eng.dma_start(t[:], src[k * 128:(k + 1) * 128, 0:512])
        ts.append(t)
    return ts


def _load_kblock_b(nc, pool, dram_ap, blk, tag, engine=None):
    """Load row-block `blk` of a host-BLOCKED tensor: one contiguous 2D DMA.
    Host layout: B[blk*128 + p, k*128 + cc] = T[k*128 + p, blk*128 + cc]."""
    t = pool.tile([128, N], F16, tag=tag)
    (engine or nc.sync).dma_start(t[:], dram_ap[blk * 128:(blk + 1) * 128, :])
    return t


def _load_kblock(nc, pool, dram_ap, col_blk, tag, dt=F16, engine=None):
    """Load DRAM[:, col_blk*128 : +128] ([N, 128]) into one [128, N] SBUF tile
    whose slice [:, k*128:(k+1)*128] is contraction-tile k (partition = row%128)."""
    t = pool.tile([128, N], dt, tag=tag)
    src = dram_ap[:, col_blk * 128:(col_blk + 1) * 128].rearrange(
        "(t p) s -> p t s", p=128
    )
    dst = t[:].rearrange("p (t s) -> p t s", t=KT)
    (engine or nc.sync).dma_start(dst, src)
    return t


def _load_resident_rest(nc, pool, src, tag):
    """Load chunks 1..3 of an [N, N] fp16 DRAM tensor as KT resident [128, N]
    row-block tiles (chunk 0 comes from the prefetch pool), on the SWDGE path."""
    ts = [pool.tile([128, N], F16, tag=f"{tag}{k}", name=f"{tag}{k}") for k in range(KT)]
    for c in range(1, NCHUNK):
        cs = slice(c * 512, (c + 1) * 512)
        for k in range(KT):
            nc.gpsimd.dma_start(ts[k][:, cs], src[k * 128:(k + 1) * 128, cs])
    return ts


def _proj_rows(nc, tc, pools, x, w, bias_bcast, out_dram, pf, nxt, ax_pre=None):
    """q/v-style projection: out[s, e] = sum_d X^T[d, s] * W^T[d, e] + bias[e].
    Chunk-outer: pass c uses only chunk c of the resident weights; chunk 0
    comes from the prefetch pool so the phase starts without waiting on the
    full weight residency. Stationary x col-blocks re-stream per pass."""
    stpool, sopool, bpool, pfpool, psum = pools
    with tc.tile_pool(name="pr_w", bufs=1) as wpool:
        wt = _load_resident_rest(nc, wpool, w, "w")
        bb = bpool.tile([128, N], F32, tag="bias")
        nc.scalar.dma_start(bb[:], bias_bcast[:])
        pf_next = None
        for c in range(NCHUNK):
            cs = slice(c * 512, (c + 1) * 512)
            for s in range(KT):
                if c == 0 and s == 0 and ax_pre is not None:
                    ax = ax_pre
                else:
                    ax = _load_kblock_b(nc, stpool, x, s, "st")
                ps = psum.tile([128, 512], F32)
                for k in range(KT):
                    mov = pf[k][:] if c == 0 else wt[k][:, cs]
                    nc.tensor.matmul(ps[:], ax[:, k * 128:(k + 1) * 128],
                                     mov, start=(k == 0), stop=(k == KT - 1))
                o16 = sopool.tile([128, 512], F16, tag="o16")
                nc.vector.tensor_add(o16[:], ps[:], bb[:, cs])
                nc.scalar.dma_start(out_dram[s * 128:(s + 1) * 128, cs], o16[:])
            if c == NCHUNK - 2 and nxt is not None:
                pf_next = _prefetch_c0(nc, pfpool, nxt[0], nxt[1])
    return pf_next


def _proj_cols_resident(nc, tc, pools, x, w, bias_part, kr, pf, nxt):
    """kT projection: kr[e][p, s] = sum_d W^T[d, e*128+p] * X^T[d, s] + bk[e*128+p].
    Chunk-outer over the s dimension; moving = resident activations (chunk 0
    prefetched), stationary = streamed weight col-blocks. Output written
    directly into resident SBUF tiles (fp16)."""
    stpool, sopool, bpool, pfpool, psum = pools
    with tc.tile_pool(name="pc_x", bufs=1) as xpool:
        xt = _load_resident_rest(nc, xpool, x, "x")
        bp = bpool.tile([128, KT], F32, tag="biasp")
        nc.scalar.dma_start(bp[:], bias_part[:])
        pf_next = None
        for c in range(NCHUNK):
            cs = slice(c * 512, (c + 1) * 512)
            for e in range(KT):
                gw = _load_kblock_b(nc, stpool, w, e, "st")
                ps = psum.tile([128, 512], F32)
                for k in range(KT):
                    mov = pf[k][:] if c == 0 else xt[k][:, cs]
                    nc.tensor.matmul(ps[:], gw[:, k * 128:(k + 1) * 128],
                                     mov, start=(k == 0), stop=(k == KT - 1))
                nc.vector.tensor_scalar_add(kr[e][:, cs], ps[:], bp[:, e:e + 1])
            if c == NCHUNK - 2 and nxt is not None:
                pf_next = _prefetch_c0(nc, pfpool, nxt[0], nxt[1])
    return pf_next


def _scores_softmax_t(nc, tc, pools, q_s, kr, att):
    """scores[i, j] = sum_m q[m, i]*kT[m, j]; row softmax; transpose each
    attn row-block into the resident attn^T tile via one batched DMA xbar op."""
    stpool, sopool, bpool, pfpool, psum = pools
    with (
        tc.tile_pool(name="sc_e", bufs=2) as epool,
        tc.tile_pool(name="sc_a", bufs=4) as apool,
        tc.tile_pool(name="sc_t", bufs=4) as tpool,
    ):
        for i in range(KT):
            qi = _load_kblock(nc, stpool, q_s, i, "st")
            pss = []
            for c in range(NCHUNK):
                cs = slice(c * 512, (c + 1) * 512)
                ps = psum.tile([128, 512], F32)
                for k in range(KT):
                    nc.tensor.matmul(ps[:], qi[:, k * 128:(k + 1) * 128],
                                     kr[k][:, cs], start=(k == 0), stop=(k == KT - 1))
                pss.append(ps)
            # row stats over the full 2048-wide row
            m4 = tpool.tile([128, NCHUNK], F32, tag="m4")
            for c in range(NCHUNK):
                nc.vector.reduce_max(m4[:, c:c + 1], pss[c][:], axis=AX)
            mx = tpool.tile([128, 1], F32, tag="mx")
            nc.vector.reduce_max(mx[:], m4[:], axis=AX)
            negm = tpool.tile([128, 1], F32, tag="negm")
            nc.scalar.mul(negm[:], mx[:], -1.0)
            e16 = epool.tile([128, N], F16, tag="e16")
            sume = tpool.tile([128, NCHUNK], F32, tag="sume")
            for c in range(NCHUNK):
                cs = slice(c * 512, (c + 1) * 512)
                nc.scalar.activation(e16[:, cs], pss[c][:], EXP,
                                     bias=negm[:], scale=1.0,
                                     accum_out=sume[:, c:c + 1])
            tot = tpool.tile([128, 1], F32, tag="tot")
            nc.vector.reduce_sum(tot[:], sume[:], axis=AX)
            rcp = tpool.tile([128, 1], F32, tag="rcp")
            nc.vector.reciprocal(rcp[:], tot[:])
            a16 = apool.tile([128, N], F16, tag="a16")
            nc.vector.tensor_scalar_mul(a16[:], e16[:], rcp[:])
            # one batched xbar transpose per row-block: writes the i-th
            # 128-column window of all 16 attn^T block-rows
            # (out[p, t, f] = in[f, t*128 + p])
            dst = att[:].rearrange("p (t x) -> p t x", t=KT)[
                :, :, i * 128:(i + 1) * 128]
            nc.sync.dma_start_transpose(dst, a16[:])


def _attn_v(nc, tc, pools, v_s, att, out):
    """out[n, i] = sum_j v[j, n] * attn[i, j]; stationary = v col-blocks
    (streamed via SWDGE), moving = resident attn^T. Chunk c only depends on
    attn row-blocks 4c..4c+3, so the first groups overlap the scores tail."""
    stpool, sopool, bpool, pfpool, psum = pools
    for n in range(KT):
        vn = _load_kblock(nc, stpool, v_s, n, "st", engine=nc.gpsimd)
        for c in range(NCHUNK):
            cs = slice(c * 512, (c + 1) * 512)
            ps = psum.tile([128, 512], F32)
            for j in range(KT):
                nc.tensor.matmul(ps[:], vn[:, j * 128:(j + 1) * 128],
                                 att[:, j * N + c * 512:j * N + (c + 1) * 512],
                                 start=(j == 0), stop=(j == KT - 1))
            o32 = sopool.tile([128, 512], F32, tag="o32")
            nc.vector.tensor_copy(o32[:], ps[:])
            nc.scalar.dma_start(out[n * 128:(n + 1) * 128, cs], o32[:])


def _block(m):
    """B[blk*128 + p, k*128 + cc] = m[k*128 + p, blk*128 + cc]."""
    return np.ascontiguousarray(
        m.reshape(KT, 128, KT, 128).transpose(2, 1, 0, 3).reshape(N, N))


def prepare_in_maps(query, key_, value, Wq, bq, Wk, bk, Wv, bv):
    query = np.asarray(query, dtype=np.float32)
    key_ = np.asarray(key_, dtype=np.float32)
    value = np.asarray(value, dtype=np.float32)
    Wq = np.asarray(Wq, dtype=np.float32)
    Wk = np.asarray(Wk, dtype=np.float32)
    Wv = np.asarray(Wv, dtype=np.float32)
    bq = np.asarray(bq, dtype=np.float32)
    bk = np.asarray(bk, dtype=np.float32)
    bv = np.asarray(bv, dtype=np.float32)

    wq16 = np.ascontiguousarray(Wq.T).astype(np.float16)
    wk16 = _block(np.ascontiguousarray(Wk.T).astype(np.float16))
    wv16 = np.ascontiguousarray(Wv.T).astype(np.float16)
    bqb = np.broadcast_to(bq, (128, N)).copy()
    bvb = np.broadcast_to(bv, (128, N)).copy()
    bkp = np.ascontiguousarray(bk.reshape(KT, 128).T)

    in_maps = []
    for b in range(B):
        in_maps.append({
            "xq": _block(np.ascontiguousarray(query[b].T).astype(np.float16)),
            "xk": np.ascontiguousarray(key_[b].T).astype(np.float16),
            "xv": _block(np.ascontiguousarray(value[b].T).astype(np.float16)),
            "wq": wq16, "wk": wk16, "wv": wv16,
            "bqb": bqb, "bkp": bkp, "bvb": bvb,
        })
    return in_maps


def get_nc():
    if "nc" not in _compiled:
        _compiled["nc"] = _build()
    return _compiled["nc"]


def kernel(query, key_, value, Wq, bq, Wk, bk, Wv, bv):
    in_maps = prepare_in_maps(query, key_, value, Wq, bq, Wk, bk, Wv, bv)
    res = run_bass_kernel_spmd(get_nc(), in_maps, core_ids=list(range(B)))
    return np.stack([res.results[b]["out"] for b in range(B)]).astype(np.float32)


if __name__ == "__main__":
    rng = np.random.default_rng(0)
    inputs = {
        "query": rng.standard_normal((B, S, D), dtype=np.float32),
        "key_": rng.standard_normal((B, S, D), dtype=np.float32),
        "value": rng.standard_normal((B, S, D), dtype=np.float32),
        "Wq": (rng.standard_normal((D, D), dtype=np.float32) / np.sqrt(D)),
        "bq": rng.standard_normal(D).astype(np.float32) * 0.01,
        "Wk": (rng.standard_normal((D, D), dtype=np.float32) / np.sqrt(D)),
        "bk": rng.standard_normal(D).astype(np.float32) * 0.01,
        "Wv": (rng.standard_normal((D, D), dtype=np.float32) / np.sqrt(D)),
        "bv": rng.standard_normal(D).astype(np.float32) * 0.01,
    }
    out = kernel(**inputs)
    print("out", out.shape, out.dtype)
